# revision 19
# baseline (speedup 1.0000x reference)
"""ComplexUnPooling2D scatter kernel for 8 Trainium2 NeuronCores.

Reference semantics: out_flat = zeros(4*n); out_flat[unpool_mat.ravel()] = inputs.ravel()
where unpool_mat[i] = 4*i + off_i, off_i in [0,4)  (2x2 maxpool argmax structure,
indices strictly increasing, batch-local).  Hence, viewing the output as [n, 4]:

    out[i, j] = inputs[i] * ((unpool_mat[i] & 3) == j)

a pure streaming elementwise expand — no indirect scatter needed.

This version targets the two real limits seen in the f32 baseline trace:
  * DMA-DDR cap ~435 GB/s per core: the f32 kernel moved 23 MiB/core.
    Here values travel as fp16 *bit patterns* (uint16), offsets as one
    int8/elem, and the output is written as fp16 pairs packed in uint32:
    2 + 1 + 8 = 11 MiB/core.  fp16 rounding gives rel err <= 2^-11 ~ 4.9e-4,
    far inside the 2e-2 gate.
  * DVE runs custom fused Specs at 1 elem/cycle: one-hot over the 4n output
    domain cost 34.8 us/core.  Packing each output *pair* of fp16s into one
    uint32 halves the stream to 2n elements -> ~17.5 us/core, fully hidden
    under the DMA.

Pair-domain placement needs no position counter at all: input element i is
read (via a 2x-broadcast AP) only at pairs d = 2i and 2i+1 — one even, one
odd — so an alternating scan alt(d) = (-1)^(d+1) discriminates them.  The
host sends a single int8 LUT of the offset,
    t8 = [-1, -2, +1, +2][off]       # sign = which pair, magnitude = which half
and the device computes, with V = 1 or 65536 selecting the fp16 half,
    out32[d] = xb[d>>1] * (eq(t8, alt) + 65536 * eq(t8, 2*alt))
All arithmetic is exact in the DVE's internal fp32 (xb < 2^16, xb<<16 has a
16-bit mantissa).  uint32 -> two little-endian fp16s, zeros in all
non-selected slots; the host upcasts fp16 -> f32 on return.

Sharding: batch dim across 8 cores (2 batches/core).  Engine layout per tile:
input DMAs ride the Activation-engine HWDGE ring, output DMAs alternate the
sync/Activation rings; DVE does one fused op per tile and nothing else.
"""
import sys

sys.path.insert(0, "/opt/trn_rl_repo")

import numpy as np

import concourse.bacc as bacc
import concourse.dve_ops as dve_ops
import concourse.mybir as mybir
import concourse.tile as tile
from concourse.bass_utils import run_bass_kernel_spmd
from concourse.dve_spec import Spec, Src0, Src1, Zero, One, eq, scan, AluOp
from concourse.dve_spec import C0 as SpecC0
from concourse.dve_spec import lower as dve_lower
from concourse.dve_uop import DveOpSpec

# Problem constants (hardcoded per contract)
B, H, W, C = 16, 64, 64, 128
OUT_SHAPE = (B, 2 * H, 2 * W, C)
N_CORES = 8
N_PER_CORE = (B // N_CORES) * H * W * C  # 1,048,576 elements
P = 128  # SBUF partitions
# Tiling: input viewed per-core as [T*P, F] (row-block major — every DMA
# walks DRAM contiguously; strided layouts measured ~40% slower per ring).
F = 1024
T = N_PER_CORE // (P * F)  # 8
assert T * P * F == N_PER_CORE

# t8 = [-1, -2, +1, +2][off]: sign picks the even/odd output pair, magnitude
# the fp16 half within it.
_T8_LUT = np.array([-1, -2, 1, 2], dtype=np.int8)

_OP_NAME = "UNPOOL_ALT_PACK_ANT"


def _register_pair_op():
    for o in dve_ops.OPS:
        if o.name == _OP_NAME:
            return o

    def _ref(in0, in1, s0, s1, imm2):
        p = in0.shape[0]
        a = in0.reshape(p, -1).astype(np.float64)
        b = in1.reshape(p, -1).astype(np.float64)
        d = np.arange(a.shape[1])
        alt = np.where(d % 2 == 0, -1.0, 1.0)[None, :]
        val = b * ((a == alt) + float(s0) * (a == 2 * alt))
        return val.astype(np.uint32)

    alt = scan(AluOp.MULTIPLY, Zero - One, init=One)  # -1, +1, -1, +1, ...
    c1 = eq(Src0, alt)
    c2 = eq(Src0, alt + alt)
    spec = Spec(body=Src1 * (c1 + c2 * SpecC0), reference=_ref)
    row = max(dve_ops._SUB_OPCODE_FOR_NAME.values()) + 1
    assert row < 0x20, row
    dve_ops._SUB_OPCODE_FOR_NAME[_OP_NAME] = row
    shas = {}
    for ver in ("v3", "v4"):
        s = DveOpSpec(
            name=_OP_NAME, opcode=row, uops=dve_lower(spec, ver=ver), rd1_en=True
        )
        shas[ver] = s.sha(ver)
    op = dve_ops.DveOp(_OP_NAME, spec, subdim=False, uops_sha=shas)
    dve_ops.OPS.append(op)
    dve_ops.CUSTOM_DVE_SPECS[_OP_NAME] = op.spec
    return op


_PAIR_OP = _register_pair_op()


def _build_program():
    # Bacc (not raw Bass): its compile() runs generate_event_semaphores,
    # which splits multi-sem waits (TRN2 allows max 1 wait per instruction).
    nc = bacc.Bacc(
        "TRN2",
        target_bir_lowering=False,
        debug=False,
        num_devices=N_CORES,
    )
    # Row-block layout.  Tiles 0 and 1 load alone (small first chunks: the
    # DVE can start ~5us earlier than with fat ones); tiles 2..7 are
    # host-packed two-per-row-block ([3P, 2F]) so they ride in 3 fat input
    # DMAs with >=2KB contiguous per-partition descriptors.  10 input DMAs
    # total, 4/6 per trigger engine.
    xbh = nc.dram_tensor("xbh", [2 * P, F], mybir.dt.uint16, kind="ExternalInput").ap()
    tth = nc.dram_tensor("tth", [2 * P, F], mybir.dt.int8, kind="ExternalInput").ap()
    xbt = nc.dram_tensor(
        "xbt", [3 * P, 2 * F], mybir.dt.uint16, kind="ExternalInput"
    ).ap()
    ttt = nc.dram_tensor(
        "ttt", [3 * P, 2 * F], mybir.dt.int8, kind="ExternalInput"
    ).ap()
    y = nc.dram_tensor("y", [T * P, 2 * F], mybir.dt.uint32, kind="ExternalOutput").ap()

    with tile.TileContext(nc) as tc:
        with (
            tc.tile_pool(name="pin", bufs=1) as pin,
            tc.tile_pool(name="pout", bufs=1) as pout,
        ):
            # Every chunk gets its own buffer (unique tag, bufs=1): no pool
            # reuse semaphores anywhere, so input prefetch never waits and
            # the out pool never back-pressures the DVE.
            #
            # DMA queues share the ~435 GB/s core cap round-robin, and each
            # queue drains its FIFO in order.  Exactly two queues, inputs
            # enqueued first on each: inputs get absolute priority over
            # outputs on their queue, and each queue's input bytes are
            # sized to finish right as its first output arrives (~13.5us),
            # leaving no bandwidth hole.
            # xb rides the Activation queue, tt the sync queue: tile 0's two
            # streams load in parallel, so the DVE starts sooner.
            xts, lts = [], []
            for t in (0, 1):
                xt = pin.tile([P, F], mybir.dt.uint16, tag=f"xbh{t}")
                nc.scalar.dma_start(out=xt[:], in_=xbh[t * P : (t + 1) * P, :])
                lt = pin.tile([P, F], mybir.dt.int8, tag=f"tth{t}")
                nc.sync.dma_start(out=lt[:], in_=tth[t * P : (t + 1) * P, :])
                xts.append(xt)
                lts.append(lt)
            for c in range(3):  # tail chunk c = tiles 2+2c, 3+2c
                rows = slice(c * P, (c + 1) * P)
                xt = pin.tile([P, 2 * F], mybir.dt.uint16, tag=f"xbt{c}")
                nc.scalar.dma_start(out=xt[:], in_=xbt[rows, :])
                lt = pin.tile([P, 2 * F], mybir.dt.int8, tag=f"ttt{c}")
                nc.sync.dma_start(out=lt[:], in_=ttt[rows, :])
                xts.append(xt)
                lts.append(lt)

            def tile_views(t):
                if t < 2:
                    return xts[t][:], lts[t][:]
                c, h = 2 + (t - 2) // 2, (t - 2) % 2
                sl = slice(h * F, (h + 1) * F)
                return xts[c][:, sl], lts[c][:, sl]

            for t in range(T):
                xv, lv = tile_views(t)
                ot = pout.tile([P, 2 * F], mybir.dt.uint32, tag=f"out{t}")
                t_b = lv.unsqueeze(2).to_broadcast([P, F, 2])
                x_b = xv.unsqueeze(2).to_broadcast([P, F, 2])
                nc.vector._custom_dve(
                    _PAIR_OP, out=ot[:], in0=t_b, in1=x_b, s0=65536.0
                )
                # Each output splits into partition halves, one per queue:
                # the two queues stay byte-balanced to the very end, always
                # behind their own inputs in FIFO order.
                r0 = t * P
                nc.scalar.dma_start(out=y[r0 : r0 + P // 2, :], in_=ot[: P // 2, :])
                nc.sync.dma_start(out=y[r0 + P // 2 : r0 + P, :], in_=ot[P // 2 :, :])
    nc.compile()
    return nc


_NC_CACHE = None


def _get_program():
    global _NC_CACHE
    if _NC_CACHE is None:
        _NC_CACHE = _build_program()
    return _NC_CACHE


def _offsets(idx: np.ndarray) -> np.ndarray:
    """off = idx & 3 via a byte-level view of the (little-endian) index words."""
    flat = np.ascontiguousarray(idx).reshape(-1)
    step = flat.dtype.itemsize  # int64 -> every 8th byte
    return np.ascontiguousarray(flat.view(np.uint8).reshape(-1, step)[:, 0]) & 3


def _make_in_maps(inputs: np.ndarray, unpool_mat: np.ndarray):
    bpc = B // N_CORES  # batches per core
    in_maps = []
    for c in range(N_CORES):
        sl = slice(c * bpc, (c + 1) * bpc)
        xb = np.ascontiguousarray(inputs[sl]).astype(np.float16).view(np.uint16)
        off = _offsets(unpool_mat[sl])
        t8 = _T8_LUT[off]

        # tiles 0-1 stay natural; tiles 2..7 pack two-per-row-block:
        # row-block k, partition p holds [tile 2k+2 row p | tile 2k+3 row p]
        def split(a):
            a = a.reshape(T * P, F)
            head = a[: 2 * P]
            tail = a[2 * P :].reshape(3, 2, P, F)
            tail = np.ascontiguousarray(tail.transpose(0, 2, 1, 3)).reshape(
                3 * P, 2 * F
            )
            return head, tail

        xbh, xbt = split(xb)
        tth, ttt = split(t8)
        in_maps.append({"xbh": xbh, "tth": tth, "xbt": xbt, "ttt": ttt})
    return in_maps


def kernel(inputs, unpool_mat, output_shape=None, **_unused):
    inputs = np.asarray(inputs)
    unpool_mat = np.asarray(unpool_mat)
    assert inputs.shape == (B, H, W, C), inputs.shape
    if output_shape is not None:
        assert tuple(int(s) for s in np.asarray(output_shape).reshape(-1)) == OUT_SHAPE

    # The fast path relies on the 2x2-maxpool-argmax structure
    # (idx[i] in [4i, 4i+4), i.e. idx >> 2 == arange) and on values fitting
    # fp16 range.  The reference generator guarantees both; verify cheaply
    # and fall back if violated.
    flat_idx = unpool_mat.reshape(-1)
    n = flat_idx.size
    structured = np.array_equal(
        flat_idx >> 2, np.arange(n, dtype=flat_idx.dtype)
    ) and np.all(np.isfinite(inputs)) and float(np.max(np.abs(inputs))) < 60000.0
    if not structured:
        out_flat = np.zeros(int(np.prod(OUT_SHAPE)), dtype=inputs.dtype)
        out_flat[flat_idx] = inputs.reshape(-1)
        return out_flat.reshape(OUT_SHAPE)

    nc = _get_program()
    in_maps = _make_in_maps(inputs, unpool_mat)
    res = run_bass_kernel_spmd(nc, in_maps, core_ids=list(range(N_CORES)))
    bpc = B // N_CORES
    out = np.concatenate(
        [
            r["y"].view(np.float16).astype(np.float32).reshape(bpc, 2 * H, 2 * W, C)
            for r in res.results
        ],
        axis=0,
    )
    return out


# revision 20
# speedup vs baseline: 1.0608x; 1.0608x over previous
"""ComplexUnPooling2D scatter kernel for 8 Trainium2 NeuronCores.

Reference semantics: out_flat = zeros(4*n); out_flat[unpool_mat.ravel()] = inputs.ravel()
where unpool_mat[i] = 4*i + off_i, off_i in [0,4)  (2x2 maxpool argmax structure,
indices strictly increasing, batch-local).  Hence, viewing the output as [n, 4]:

    out[i, j] = inputs[i] * ((unpool_mat[i] & 3) == j)

a pure streaming elementwise expand — no indirect scatter needed.

This version targets the two real limits seen in the f32 baseline trace:
  * DMA-DDR cap ~435 GB/s per core: the f32 kernel moved 23 MiB/core.
    Here values travel as fp16 *bit patterns* (uint16), offsets as one
    int8/elem, and the output is written as fp16 pairs packed in uint32:
    2 + 1 + 8 = 11 MiB/core.  fp16 rounding gives rel err <= 2^-11 ~ 4.9e-4,
    far inside the 2e-2 gate.
  * DVE runs custom fused Specs at 1 elem/cycle: one-hot over the 4n output
    domain cost 34.8 us/core.  Packing each output *pair* of fp16s into one
    uint32 halves the stream to 2n elements -> ~17.5 us/core, fully hidden
    under the DMA.

Pair-domain placement needs no position counter at all: input element i is
read (via a 2x-broadcast AP) only at pairs d = 2i and 2i+1 — one even, one
odd — so an alternating scan alt(d) = (-1)^(d+1) discriminates them.  The
host sends a single int8 LUT of the offset,
    t8 = [-1, -2, +1, +2][off]       # sign = which pair, magnitude = which half
and the device computes, with V = 1 or 65536 selecting the fp16 half,
    out32[d] = xb[d>>1] * (eq(t8, alt) + 65536 * eq(t8, 2*alt))
All arithmetic is exact in the DVE's internal fp32 (xb < 2^16, xb<<16 has a
16-bit mantissa).  uint32 -> two little-endian fp16s, zeros in all
non-selected slots; the host upcasts fp16 -> f32 on return.

Sharding: batch dim across 8 cores (2 batches/core).  Engine layout per tile:
input DMAs ride the Activation-engine HWDGE ring, output DMAs alternate the
sync/Activation rings; DVE does one fused op per tile and nothing else.
"""
import sys

sys.path.insert(0, "/opt/trn_rl_repo")

import numpy as np

import concourse.bacc as bacc
import concourse.dve_ops as dve_ops
import concourse.mybir as mybir
import concourse.tile as tile
from concourse.bass_utils import run_bass_kernel_spmd
from concourse.dve_spec import Spec, Src0, Src1, Zero, One, eq, scan, AluOp
from concourse.dve_spec import C0 as SpecC0
from concourse.dve_spec import lower as dve_lower
from concourse.dve_uop import DveOpSpec

# Problem constants (hardcoded per contract)
B, H, W, C = 16, 64, 64, 128
OUT_SHAPE = (B, 2 * H, 2 * W, C)
N_CORES = 8
N_PER_CORE = (B // N_CORES) * H * W * C  # 1,048,576 elements
P = 128  # SBUF partitions
# Tiling: input viewed per-core as [T*P, F] (row-block major — every DMA
# walks DRAM contiguously; strided layouts measured ~40% slower per ring).
F = 1024
T = N_PER_CORE // (P * F)  # 8
assert T * P * F == N_PER_CORE

# t8 = [-1, -2, +1, +2][off]: sign picks the even/odd output pair, magnitude
# the fp16 half within it.
_T8_LUT = np.array([-1, -2, 1, 2], dtype=np.int8)

_OP_NAME = "UNPOOL_ALT_PACK_ANT"


def _register_pair_op():
    for o in dve_ops.OPS:
        if o.name == _OP_NAME:
            return o

    def _ref(in0, in1, s0, s1, imm2):
        p = in0.shape[0]
        a = in0.reshape(p, -1).astype(np.float64)
        b = in1.reshape(p, -1).astype(np.float64)
        d = np.arange(a.shape[1])
        alt = np.where(d % 2 == 0, -1.0, 1.0)[None, :]
        val = b * ((a == alt) + float(s0) * (a == 2 * alt))
        return val.astype(np.uint32)

    alt = scan(AluOp.MULTIPLY, Zero - One, init=One)  # -1, +1, -1, +1, ...
    c1 = eq(Src0, alt)
    c2 = eq(Src0, alt + alt)
    spec = Spec(body=Src1 * (c1 + c2 * SpecC0), reference=_ref)
    row = max(dve_ops._SUB_OPCODE_FOR_NAME.values()) + 1
    assert row < 0x20, row
    dve_ops._SUB_OPCODE_FOR_NAME[_OP_NAME] = row
    shas = {}
    for ver in ("v3", "v4"):
        s = DveOpSpec(
            name=_OP_NAME, opcode=row, uops=dve_lower(spec, ver=ver), rd1_en=True
        )
        shas[ver] = s.sha(ver)
    op = dve_ops.DveOp(_OP_NAME, spec, subdim=False, uops_sha=shas)
    dve_ops.OPS.append(op)
    dve_ops.CUSTOM_DVE_SPECS[_OP_NAME] = op.spec
    return op


_PAIR_OP = _register_pair_op()


def _build_program():
    # Bacc (not raw Bass): its compile() runs generate_event_semaphores,
    # which splits multi-sem waits (TRN2 allows max 1 wait per instruction).
    nc = bacc.Bacc(
        "TRN2",
        target_bir_lowering=False,
        debug=False,
        num_devices=N_CORES,
    )
    # Row-block layout.  Tiles 0 and 1 load alone (small first chunks: the
    # DVE can start ~5us earlier than with fat ones); tiles 2..7 are
    # host-packed two-per-row-block ([3P, 2F]) so they ride in 3 fat input
    # DMAs with >=2KB contiguous per-partition descriptors.  10 input DMAs
    # total, 4/6 per trigger engine.
    xbh = nc.dram_tensor("xbh", [2 * P, F], mybir.dt.uint16, kind="ExternalInput").ap()
    tth = nc.dram_tensor("tth", [2 * P, F], mybir.dt.int8, kind="ExternalInput").ap()
    xbt = nc.dram_tensor(
        "xbt", [3 * P, 2 * F], mybir.dt.uint16, kind="ExternalInput"
    ).ap()
    ttt = nc.dram_tensor(
        "ttt", [3 * P, 2 * F], mybir.dt.int8, kind="ExternalInput"
    ).ap()
    y = nc.dram_tensor("y", [T * P, 2 * F], mybir.dt.uint32, kind="ExternalOutput").ap()

    with tile.TileContext(nc) as tc:
        with (
            tc.tile_pool(name="pin", bufs=1) as pin,
            tc.tile_pool(name="pout", bufs=1) as pout,
        ):
            # Every chunk gets its own buffer (unique tag, bufs=1): no pool
            # reuse semaphores anywhere, so input prefetch never waits and
            # the out pool never back-pressures the DVE.
            #
            # DMA queues share the ~435 GB/s core cap round-robin, and each
            # queue drains its FIFO in order.  Exactly two queues, inputs
            # enqueued first on each: inputs get absolute priority over
            # outputs on their queue, and each queue's input bytes are
            # sized to finish right as its first output arrives (~13.5us),
            # leaving no bandwidth hole.
            # xb rides the Activation queue, tt the sync queue: tile 0's two
            # streams load in parallel, so the DVE starts sooner.
            xts, lts = [], []
            for t in (0, 1):
                xt = pin.tile([P, F], mybir.dt.uint16, tag=f"xbh{t}")
                nc.scalar.dma_start(out=xt[:], in_=xbh[t * P : (t + 1) * P, :])
                lt = pin.tile([P, F], mybir.dt.int8, tag=f"tth{t}")
                nc.sync.dma_start(out=lt[:], in_=tth[t * P : (t + 1) * P, :])
                xts.append(xt)
                lts.append(lt)
            for c in range(3):  # tail chunk c = tiles 2+2c, 3+2c
                rows = slice(c * P, (c + 1) * P)
                xt = pin.tile([P, 2 * F], mybir.dt.uint16, tag=f"xbt{c}")
                nc.scalar.dma_start(out=xt[:], in_=xbt[rows, :])
                lt = pin.tile([P, 2 * F], mybir.dt.int8, tag=f"ttt{c}")
                nc.sync.dma_start(out=lt[:], in_=ttt[rows, :])
                xts.append(xt)
                lts.append(lt)

            def tile_views(t):
                if t < 2:
                    return xts[t][:], lts[t][:]
                c, h = 2 + (t - 2) // 2, (t - 2) % 2
                sl = slice(h * F, (h + 1) * F)
                return xts[c][:, sl], lts[c][:, sl]

            for t in range(T):
                xv, lv = tile_views(t)
                ot = pout.tile([P, 2 * F], mybir.dt.uint32, tag=f"out{t}")
                t_b = lv.unsqueeze(2).to_broadcast([P, F, 2])
                x_b = xv.unsqueeze(2).to_broadcast([P, F, 2])
                nc.vector._custom_dve(
                    _PAIR_OP, out=ot[:], in0=t_b, in1=x_b, s0=65536.0
                )
                # Full-width outputs alternating the two queues (half-DMAs
                # measured ~15% lower aggregate efficiency), always behind
                # their own queue's inputs in FIFO order.
                oeng = nc.scalar if t % 2 == 0 else nc.sync
                oeng.dma_start(out=y[t * P : (t + 1) * P, :], in_=ot[:])
    nc.compile()
    return nc


_NC_CACHE = None


def _get_program():
    global _NC_CACHE
    if _NC_CACHE is None:
        _NC_CACHE = _build_program()
    return _NC_CACHE


def _offsets(idx: np.ndarray) -> np.ndarray:
    """off = idx & 3 via a byte-level view of the (little-endian) index words."""
    flat = np.ascontiguousarray(idx).reshape(-1)
    step = flat.dtype.itemsize  # int64 -> every 8th byte
    return np.ascontiguousarray(flat.view(np.uint8).reshape(-1, step)[:, 0]) & 3


def _make_in_maps(inputs: np.ndarray, unpool_mat: np.ndarray):
    bpc = B // N_CORES  # batches per core
    in_maps = []
    for c in range(N_CORES):
        sl = slice(c * bpc, (c + 1) * bpc)
        xb = np.ascontiguousarray(inputs[sl]).astype(np.float16).view(np.uint16)
        off = _offsets(unpool_mat[sl])
        t8 = _T8_LUT[off]

        # tiles 0-1 stay natural; tiles 2..7 pack two-per-row-block:
        # row-block k, partition p holds [tile 2k+2 row p | tile 2k+3 row p]
        def split(a):
            a = a.reshape(T * P, F)
            head = a[: 2 * P]
            tail = a[2 * P :].reshape(3, 2, P, F)
            tail = np.ascontiguousarray(tail.transpose(0, 2, 1, 3)).reshape(
                3 * P, 2 * F
            )
            return head, tail

        xbh, xbt = split(xb)
        tth, ttt = split(t8)
        in_maps.append({"xbh": xbh, "tth": tth, "xbt": xbt, "ttt": ttt})
    return in_maps


def kernel(inputs, unpool_mat, output_shape=None, **_unused):
    inputs = np.asarray(inputs)
    unpool_mat = np.asarray(unpool_mat)
    assert inputs.shape == (B, H, W, C), inputs.shape
    if output_shape is not None:
        assert tuple(int(s) for s in np.asarray(output_shape).reshape(-1)) == OUT_SHAPE

    # The fast path relies on the 2x2-maxpool-argmax structure
    # (idx[i] in [4i, 4i+4), i.e. idx >> 2 == arange) and on values fitting
    # fp16 range.  The reference generator guarantees both; verify cheaply
    # and fall back if violated.
    flat_idx = unpool_mat.reshape(-1)
    n = flat_idx.size
    structured = np.array_equal(
        flat_idx >> 2, np.arange(n, dtype=flat_idx.dtype)
    ) and np.all(np.isfinite(inputs)) and float(np.max(np.abs(inputs))) < 60000.0
    if not structured:
        out_flat = np.zeros(int(np.prod(OUT_SHAPE)), dtype=inputs.dtype)
        out_flat[flat_idx] = inputs.reshape(-1)
        return out_flat.reshape(OUT_SHAPE)

    nc = _get_program()
    in_maps = _make_in_maps(inputs, unpool_mat)
    res = run_bass_kernel_spmd(nc, in_maps, core_ids=list(range(N_CORES)))
    bpc = B // N_CORES
    out = np.concatenate(
        [
            r["y"].view(np.float16).astype(np.float32).reshape(bpc, 2 * H, 2 * W, C)
            for r in res.results
        ],
        axis=0,
    )
    return out
